# revision 36
# baseline (speedup 1.0000x reference)
"""NEAT layer kernel for Trainium2 (8 NeuronCores, pure data parallel).

Math (per reference): vals starts as x [B,64]; for each layer li with
(src, w, b): z = sum_k vals[:, src[n,k]] * w[n,k] + b[n]; out = sigmoid(5*z);
vals = concat(vals, out). Output = layer-3 out [B,10].

Design (v2): the sparse gather+einsum is a chain of dense matmuls with
host-scattered weights. All matmul operands are float16 (the PE streams
one 512-column moving tile in ~216 ns warm, dtype-independent down to
16-bit; fp32/fp32r streams at half rate). Batch 65536 -> 8192/core ->
16 chunks of 512 columns (one PSUM bank per z-tile).

PSUM bank packing (the ACT engine at ~(N+300)/1.2 ns per call is the
second bottleneck, so sigmoid work is packed into as few 512-col bank
reads as possible):
  A(c)  = z0(c) rows 0..127                       (1 bank/chunk)
  D(c)  = { z1'(c) rows 0..n1-1, z3partial(c) rows n1..n1+9,
            zeros.., z2(c-2) rows 96..127 }       (1 bank/chunk)
  R(j)  = { q3(4j+r) rows 32r..32r+9, r=0..3 }    (1 bank/4 chunks)
z1 is pruned to the n1 (=70 for this topology) layer-1 nodes actually
referenced downstream. Regions co-exist in one bank via the has_written
accumulation bits: the first writer of a bank generation uses start=True
and a full 128-wide stationary (zero-padded), defining every row;
later region-groups use start=False with zero columns elsewhere, which
accumulates +0 outside their region. PSUM dst base partitions stay at 0
(col quadrant 3, base 96, hard-crashes the device; zero-padding to
M=128 writes high rows without an explicit 96 base).

ACT calls per chunk: A (512 cols, bias0), D (512 cols, mixed per-row
bias vector), R amortized /4. The z3 partial rows get sigmoid'd garbage
in the fp16 staging tile and are overwritten with the raw psum values
by a DVE copy; L3 folds them through identity rows in its stationary.
All stationaries are padded to K>=65 / M=128 so every matmul has
tile_size (128,128) -- no PE tiling-mode switches.

Chunk-c step emits: L0(c+1), ACT-A(c+1) (hoisted one step so the
sigmoid is off the L1 critical path), L1x/L1h(c) -> D(c)[0:96+],
L2x/L2h0/L2h1(c-2) -> D(c)[96:128], ACT-D(c), DVE z3p copy,
L3a/L3b(c-4) -> R, ACT-R every 4. o2(c-2) lands at column block c of
the staging tile (lane-aligned ACT), so L3b reads column c+2.

x travels as a [65, 8192] fp16 blob (row 64 zero-pad for K=65 ->
tile_size 128) in 4 quarter DMAs overlapped with compute; weights in
one fp16 blob; biases (pre-scaled by 5) in a small fp32 tensor read as
per-partition ACT bias APs.
"""

import sys

sys.path.insert(0, "/opt/trn_rl_repo")

import numpy as np

import concourse.bass as bass
import concourse.mybir as mybir
from concourse.tile import TileContext

BATCH = 65536
IN_DIM = 64
FAN_IN = 16
GAIN = 5.0
N_CORES = 8
BC = BATCH // N_CORES          # 8192 samples per core
CHUNK = 512
NC_CH = BC // CHUNK            # 16 chunks
NQ = 2                         # x DMA pieces (4 in-DMAs + 4 out = 8 lanes)

# Node index blocks in the accumulated `vals` array.
X_LO, X_HI = 0, 64
H0_LO, H0_HI = 64, 192
H1_LO, H1_HI = 192, 288
H2_LO, H2_HI = 288, 320

F32 = mybir.dt.float32
F16 = mybir.dt.float16
SIG = mybir.ActivationFunctionType.Sigmoid


def _needed_sets(inputs):
    """Layer-1 nodes referenced by later layers (pruning set), in order."""
    src2 = np.asarray(inputs["src2"])
    src3 = np.asarray(inputs["src3"])
    cand = np.concatenate([src2.ravel(), src3.ravel()])
    need1 = np.unique(cand[(cand >= H1_LO) & (cand < H1_HI)])
    return need1


def _scatter(dst, src, w, lo, hi, col_off, col_map=None):
    """dst[src[n,k]-lo, cmap(n)+col_off] += w[n,k] for src in [lo,hi)."""
    n, k = src.shape
    cols = np.repeat(np.arange(n, dtype=np.int64), k)
    s = np.asarray(src).ravel().astype(np.int64)
    v = np.asarray(w).ravel().astype(np.float64)
    m = (s >= lo) & (s < hi)
    c = cols[m]
    if col_map is not None:
        c = col_map[c]
        keep = c >= 0
        sm, cm, vm = s[m][keep], c[keep], v[m][keep]
    else:
        sm, cm, vm = s[m], c, v[m]
    np.add.at(dst, (sm - lo, cm + col_off), vm)


class Plan:
    """Host-side layout plan (data-dependent via z1 pruning)."""

    def __init__(self, inputs):
        need1 = _needed_sets(inputs)
        n1 = len(need1)
        if n1 + 10 > 96:
            # Fallback: no pruning possible within the packed-bank layout;
            # keep full z1 (96) and accept z3p at rows 96..105 is impossible
            # -> place z3p in rows 96..105? Not available. Use first 86.
            # (Never hits for this problem's topology; guard anyway.)
            raise NotImplementedError("n1 > 86 not supported")
        self.need1 = need1
        self.n1 = n1
        # D-bank rows. z3p must sit at an aligned base (engine SBUF APs
        # start at 0/32/64/96 only), so z1' columns split around it:
        # o1 node i -> row i (i<64) or row 74+(i-64); z3p rows 64..73.
        self.p3_lo = 64
        o1_rows = np.arange(n1)
        o1_rows = np.where(o1_rows < 64, o1_rows, o1_rows + 10)
        assert o1_rows.max(initial=0) < 96
        self.o1_rows = o1_rows
        self.z2_lo = 96                     # rows [96, 128)

    def build_weights(self, inputs):
        """fp16 weight blob [128, WCOLS] + fp32 bias [128, 4]."""
        n1 = self.n1
        need1 = self.need1
        # col_map for layer-1 node -> D-bank row (split around z3p rows)
        cmap1 = np.full(H1_HI - H1_LO, -1, dtype=np.int64)
        cmap1[need1 - H1_LO] = self.o1_rows

        # Stationaries (fp64 accum then fp16). All M=128 wide; the x
        # matmuls are K=64 so {L0,L1x} can row-tile as a concurrent pair.
        WX0 = np.zeros([64, 128], np.float64)    # x -> z0
        WX1 = np.zeros([64, 128], np.float64)    # x -> z1' + z3p (cols 64..73)
        WH1 = np.zeros([128, 128], np.float64)   # o0 -> z1' + z3p
        WX2 = np.zeros([64, 128], np.float64)    # x -> z2 (cols 96..127)
        WH20 = np.zeros([128, 128], np.float64)  # o0 -> z2
        WH21 = np.zeros([128, 128], np.float64)  # o1' -> z2 (rows 0..n1-1)
        # L3: one [128, 224] strip; residue-r window cols [96-32r, 224-32r)
        # puts the q3 weights (absolute cols 96..105) at window cols
        # 32r..32r+9 -> psum rows 32r..32r+9.
        W3S = np.zeros([128, 224], np.float64)

        _scatter(WX0, inputs["src0"], inputs["w0"], X_LO, X_HI, 0)

        _scatter(WX1, inputs["src1"], inputs["w1"], X_LO, X_HI, 0,
                 col_map=cmap1)
        _scatter(WH1, inputs["src1"], inputs["w1"], H0_LO, H0_HI, 0,
                 col_map=cmap1)
        # z3 partial (x + o0 contributions) in cols 64..73
        _scatter(WX1, inputs["src3"], inputs["w3"], X_LO, X_HI, self.p3_lo)
        _scatter(WH1, inputs["src3"], inputs["w3"], H0_LO, H0_HI, self.p3_lo)

        _scatter(WX2, inputs["src2"], inputs["w2"], X_LO, X_HI, 96)
        _scatter(WH20, inputs["src2"], inputs["w2"], H0_LO, H0_HI, 96)
        # o1' rows of WH21: scatter by pruned row index
        s2 = np.asarray(inputs["src2"])
        w2 = np.asarray(inputs["w2"])
        m = (s2 >= H1_LO) & (s2 < H1_HI)
        rows = cmap1[s2[m] - H1_LO]
        cols = np.repeat(np.arange(s2.shape[0], dtype=np.int64),
                         s2.shape[1])[m.ravel()] + 96
        np.add.at(WH21, (rows, cols), w2[m].astype(np.float64))

        # W3S weight cols at absolute 96..105: o1' rows, z3p identity fold,
        # o2 rows (the DVE shift aligns o2 to the same td column).
        s3 = np.asarray(inputs["src3"])
        w3 = np.asarray(inputs["w3"])
        m = (s3 >= H1_LO) & (s3 < H1_HI)
        rows = cmap1[s3[m] - H1_LO]
        cols = np.repeat(np.arange(10, dtype=np.int64), FAN_IN)[m.ravel()]
        np.add.at(W3S, (rows, cols + 96), w3[m].astype(np.float64))
        W3S[np.arange(self.p3_lo, self.p3_lo + 10),
            np.arange(96, 106)] = 1.0  # identity fold for z3 partial
        m = (s3 >= H2_LO) & (s3 < H2_HI)
        rows = (s3[m] - H2_LO) + 96
        cols = np.repeat(np.arange(10, dtype=np.int64), FAN_IN)[m.ravel()]
        np.add.at(W3S, (rows, cols + 96), w3[m].astype(np.float64))

        # Blob layout: WX0 on rows 0..63 and WX1 on rows 64..127 share
        # columns (L0 reads the low half, the row-tiled L1x the high).
        ncols = 128 * 5 + 224
        blob = np.zeros([128, ncols], np.float16)
        offs = {}
        blob[0:64, 0:128] = WX0.astype(np.float16)
        blob[64:128, 0:128] = WX1.astype(np.float16)
        offs["WX0"] = 0
        offs["WX1"] = 0
        o = 128
        for nm, m_ in [("WX2", WX2), ("WH1", WH1), ("WH20", WH20),
                       ("WH21", WH21), ("W3S", W3S)]:
            blob[0:m_.shape[0], o:o + m_.shape[1]] = m_.astype(np.float16)
            offs[nm] = o
            o += m_.shape[1]
        self.offs = offs
        self.wcols = ncols

        bias = np.zeros([128, 4], np.float32)
        bias[:, 0] = GAIN * np.asarray(inputs["b0"], np.float32)
        bias[self.o1_rows, 1] = GAIN * np.asarray(inputs["b1"], np.float32)[
            need1 - H1_LO]
        bias[96:128, 1] = GAIN * np.asarray(inputs["b2"], np.float32)
        for r in range(4):
            bias[32 * r:32 * r + 10, 2] = GAIN * np.asarray(
                inputs["b3"], np.float32)
        return blob, bias


X_PIECES = [16]                # single x DMA (only L0(0) waits it)
Y_PIECES = [2, 1, 1]           # R-groups per out DMA piece


def build_nc(n1: int, offs: dict, wcols: int) -> bass.Bass:
    nc = bass.Bass()
    WCOLS = wcols
    wblob = nc.declare_dram_parameter("wblob", [128, WCOLS], F16,
                                      isOutput=False)
    bias_d = nc.declare_dram_parameter("bias", [128, 4], F32, isOutput=False)
    xq = [nc.declare_dram_parameter(f"x{q}", [128, p * CHUNK], F16,
                                    isOutput=False)
          for q, p in enumerate(X_PIECES)]
    # Output: full columns of the R staging tile, host slices rows.
    yT = [nc.declare_dram_parameter(f"yT{j}", [128, p * CHUNK], F16,
                                    isOutput=True)
          for j, p in enumerate(Y_PIECES)]

    with TileContext(nc) as tc:
        with (
            tc.tile_pool(name="persist", bufs=1) as pp,
            tc.tile_pool(name="pa", bufs=2, space="PSUM") as pa,
            tc.tile_pool(name="pd", bufs=2, space="PSUM") as pd,
            tc.tile_pool(name="pr", bufs=2, space="PSUM") as pr,
        ):
            w_sb = pp.tile([128, WCOLS], F16)
            bias_sb = pp.tile([128, 4], F32)
            # x on rows 0..63 and duplicated on rows 64..127 so the
            # row-tiled L1x (tile 64,0) can stream it from the high half.
            x_sb = pp.tile([128, BC], F16)
            ta = pp.tile([128, BC], F16)     # o0
            # o1'/z3p at col c; z2(c-3)'s raw o2 landing also at col c
            td = pp.tile([128, (NC_CH + 3) * CHUNK], F16)
            tr = pp.tile([128, (NC_CH // 4) * CHUNK], F16)  # q3 slots
            warm = pp.tile([128, 1], F32)

            nc.sync.dma_start(out=w_sb[:], in_=wblob[:])
            nc.sync.dma_start(out=bias_sb[:], in_=bias_d[:])
            co = 0
            for q, p in enumerate(X_PIECES):
                nc.sync.dma_start(
                    out=x_sb[:, co * CHUNK:(co + p) * CHUNK], in_=xq[q][:])
                co += p
            # Warmups: put the wblob wait into the PE clock and the bias
            # wait into the ACT clock, so later instructions carry at most
            # one sync wait each.
            nc.tensor.ldweights(w_sb[0:128, 0:128])
            nc.scalar.copy(warm[:], bias_sb[:, 3:4])
            # HAM pre-warm: ~3.5us of dummy matmuls (gated only on the
            # wblob DMA) flip the PE clock gate to 8/8 before real work
            # and overlap the x DMAs.
            wtile = pr.tile([128, CHUNK], F32, name="R")
            for _ in range(20):
                nc.tensor.matmul(wtile[:], w_sb[0:128, 0:128],
                                 w_sb[0:128, 0:CHUNK], start=True, stop=True)
            # Consume the prewarm bank with a tiny ACT read so its later
            # recycle dep is an (old) Activation value covered by the PE
            # engine clock, keeping that L0 at one sync wait.
            warm2 = pp.tile([128, 1], F32)
            nc.scalar.copy(warm2[:], wtile[:, 0:1])

            def W(name):
                o = offs[name]
                return w_sb[0:128, o:o + 128]

            def xsl(c):
                return x_sb[0:64, c * CHUNK:(c + 1) * CHUNK]

            def xsh(c):
                return x_sb[64:128, c * CHUNK:(c + 1) * CHUNK]

            def cols(t, c):
                return t[:, c * CHUNK:(c + 1) * CHUNK]

            d_gens, r_banks = {}, {}
            r_started = {}
            NB_D = NC_CH + 3        # D banks 0..18

            def Dk(k):
                j = k // 2
                if j not in d_gens:
                    d_gens[j] = pd.tile([128, 2 * CHUNK], F32, name="Dp")
                g = d_gens[j]
                return g[:, (k % 2) * CHUNK:(k % 2 + 1) * CHUNK]

            # Steady-state software pipeline. At step t:
            #   L0(t)+ACT-A(t) | L1grp(t-1)->D(t-1) | L2grp(t-3)->D(t)
            #   | even t: pair ACT-D{D(t-2),D(t-1)} | DVE shifts+casts
            #   | L3(t-6)->R | ACT-R every 4.
            # Bank D(k) holds {z1'(k),z3p(k), z2(k-3)}; its first writer
            # is L2x(k-3) (start=True, 128-wide) for k>=3 else L1x(k).
            for t in range(NC_CH + 6):
                c0 = t
                c1 = t - 1
                c2 = t - 3
                c3 = t - 6

                if c0 < NC_CH:
                    A = pa.tile([128, CHUNK], F32, name="A")
                    nc.tensor.matmul(A[:], w_sb[0:64, 0:128], xsl(c0),
                                     start=True, stop=True)
                    nc.scalar.activation(cols(ta, c0), A[:], SIG,
                                         bias=bias_sb[:, 0:1], scale=GAIN)

                if 0 <= c1 < NC_CH:
                    # L1 group (row-tiled L1x streams the high x copy
                    # concurrently with the next chunk's L0).
                    D = Dk(c1)
                    nc.tensor.matmul(D, w_sb[64:128, 0:128], xsh(c1),
                                     start=(c1 < 3), stop=False)
                    nc.tensor.matmul(D, W("WH1"), cols(ta, c1),
                                     start=False, stop=True)
                if 0 <= c2 < NC_CH:
                    # L2 group -> rows 96..127 of D(c2+3): the bank's
                    # first writer (128-wide start=True zero-defines it).
                    D2 = Dk(c2 + 3)
                    nc.tensor.matmul(D2, w_sb[0:64, 128:256], xsl(c2),
                                     start=True, stop=False)
                    nc.tensor.matmul(D2, W("WH20"), cols(ta, c2),
                                     start=False, stop=False)
                    nc.tensor.matmul(D2, W("WH21"), cols(td, c2),
                                     start=False, stop=(c2 + 3 >= NC_CH))

                if t % 2 == 0 and 2 <= t <= NB_D + 1:
                    # Pair ACT over banks D(t-2), D(t-1) (one [128,1024]
                    # AP across the gen's two contiguous banks); the last
                    # odd bank gets a single call.
                    j = (t - 2) // 2
                    g = d_gens[j]
                    n = 2 * CHUNK if t - 1 < NB_D else CHUNK
                    nc.scalar.activation(
                        td[:, (t - 2) * CHUNK:(t - 2) * CHUNK + n],
                        g[:, 0:n], SIG, bias=bias_sb[:, 1:2], scale=GAIN)
                    # DVE: shifts first (keeps WAR values covered), then
                    # z3p raw-psum casts.
                    for k in (t - 2, t - 1):
                        if 3 <= k < NB_D and k - 3 < NC_CH:
                            nc.vector.tensor_copy(
                                cols(td, k - 3)[96:128, :],
                                cols(td, k)[96:128, :])
                    for k in (t - 2, t - 1):
                        if k < NC_CH:
                            nc.vector.tensor_copy(
                                cols(td, k)[64:74, :], Dk(k)[64:74, :])

                if 0 <= c3 < NC_CH:
                    j, r = divmod(c3, 4)
                    if r == 0:
                        r_banks[j] = pr.tile([128, CHUNK], F32, name="R")
                        r_started[j] = False
                    R = r_banks[j]
                    o3s = offs["W3S"]
                    nc.tensor.matmul(
                        R[:],
                        w_sb[0:128, o3s + 96 - 32 * r:o3s + 224 - 32 * r],
                        cols(td, c3),
                        start=not r_started[j], stop=(r == 3))
                    r_started[j] = True
                    if r == 3:
                        nc.scalar.activation(
                            tr[:, j * CHUNK:(j + 1) * CHUNK], R[:], SIG,
                            bias=bias_sb[:, 2:3], scale=GAIN)
                        jo = [0, 0, 1, 2][j]
                        if j in (1, 2, 3):
                            base = [0, 0, 2, 3][j]
                            nc.sync.dma_start(
                                out=yT[jo][:],
                                in_=tr[:, base * CHUNK:(j + 1) * CHUNK])

    _prune_sync(nc)
    return nc


def _prune_sync(nc):
    """Two sound wait prunes keeping every instruction at <=1 sync wait.

    (1) Matmults waiting {Activation, DVE}: the DVE copy for column c runs
    after ACT-D(c) (it reads the same PSUM bank), so a DVE-lane wait of
    value d implies the Activation lane reached the value the DVE op
    itself waited on (or an earlier DVE op did).  Drop the Activation
    wait when its value is <= that cover (checked per instruction).

    (2) The teardown Drain waits every engine lane; the last out-DMA's
    completion transitively dominates them all.
    """
    insts = list(nc.all_instructions())
    # implied[lane][value] = {other_lane: min value other_lane must have
    # reached once `lane` reaches `value`} -- built by walking the
    # scheduled instruction stream and propagating each instruction's
    # waits (plus their implications) into the values it updates.
    implied = {}
    cum = {}
    for i in insts:
        si = i.sync_info
        if not si or not si.on_update:
            continue
        imp = {}
        for w in si.on_wait:
            imp[w.ant_name] = max(imp.get(w.ant_name, -1), w.wait_value)
            for ln, v in implied.get(w.ant_name, {}).get(
                    w.wait_value, {}).items():
                imp[ln] = max(imp.get(ln, -1), v)
        for u in si.on_update:
            ln = u.ant_name
            cum[ln] = cum.get(ln, 0) + (getattr(u, "update_value", 1) or 1)
            d = implied.setdefault(ln, {})
            # An update implies everything earlier updates on this lane
            # implied (engine program order), merged with this inst's.
            prev = d.get(max(d.keys(), default=None), {}) if d else {}
            merged = dict(prev)
            for k, v in imp.items():
                merged[k] = max(merged.get(k, -1), v)
            d[cum[ln]] = merged
    # Engine clocks: per engine (keyed by the lane it updates; all DMA
    # queues share one FIFO), the lane values its earlier instructions
    # have already waited past (directly or transitively). A wait at or
    # below the engine clock is redundant -- the engine dispatches in
    # order, so the earlier wait already gates this instruction.
    eng_clock = {}
    for i in insts:
        si = i.sync_info
        if not si or not si.on_update:
            continue
        ln0 = si.on_update[0].ant_name
        eng = "DMA" if ln0.startswith("DMAHW") else ln0
        clk = eng_clock.setdefault(eng, {})
        if si.on_wait:
            kept = [w for w in si.on_wait
                    if w.wait_value > clk.get(w.ant_name, -1)]
            if len(kept) >= 1 and len(kept) < len(si.on_wait):
                si.on_wait = kept
                i.sync_info = si
            for w in si.on_wait:
                clk[w.ant_name] = max(clk.get(w.ant_name, -1), w.wait_value)
                for lnn, v in implied.get(w.ant_name, {}).get(
                        w.wait_value, {}).items():
                    clk[lnn] = max(clk.get(lnn, -1), v)
    for i in insts:
        t = type(i).__name__
        si = i.sync_info
        if not si or len(si.on_wait) <= 1:
            continue
        if t == "InstDrain":
            dma_lane = None
            for j in insts:
                if type(j).__name__ == "InstDMACopy" and j.sync_info:
                    for u in j.sync_info.on_update:
                        if j.sync_info.on_wait:
                            dma_lane = u.ant_name
            si.on_wait = [w for w in si.on_wait if w.ant_name == dma_lane]
            i.sync_info = si
            continue
        ws = list(si.on_wait)
        # Greedy: drop any wait implied by another kept wait.
        changed = True
        while changed and len(ws) > 1:
            changed = False
            for k, w in enumerate(ws):
                others = [o for j2, o in enumerate(ws) if j2 != k]
                for o in others:
                    iv = implied.get(o.ant_name, {}).get(o.wait_value, {})
                    if iv.get(w.ant_name, -1) >= w.wait_value:
                        ws.pop(k)
                        changed = True
                        break
                if changed:
                    break
        si.on_wait = ws
        i.sync_info = si


def audit(nc):
    bad = []
    for i in nc.all_instructions():
        if i.sync_info and len(i.sync_info.on_wait) > 1:
            bad.append((type(i).__name__, i.name,
                        [w.ant_name for w in i.sync_info.on_wait]))
    return bad


def make_in_maps(inputs, plan, wb, bias):
    x = np.asarray(inputs["x"], np.float32)
    in_maps = []
    for i in range(N_CORES):
        m = {"wblob": wb, "bias": bias}
        xT = np.empty([128, BC], np.float16)
        xT[0:64, :] = x[i * BC:(i + 1) * BC, :].T.astype(np.float16)
        xT[64:128, :] = xT[0:64, :]
        co = 0
        for q, p in enumerate(X_PIECES):
            m[f"x{q}"] = np.ascontiguousarray(
                xT[:, co * CHUNK:(co + p) * CHUNK])
            co += p
        in_maps.append(m)
    return in_maps


def assemble_output(results):
    y = np.empty((BATCH, 10), np.float32)
    for i in range(N_CORES):
        res = results[i]
        tr = np.concatenate(
            [np.asarray(res[f"yT{k}"], np.float32)
             for k in range(len(Y_PIECES))], axis=1)  # [128, 2048]
        for j in range(NC_CH // 4):
            t = tr[:, j * CHUNK:(j + 1) * CHUNK]
            for r in range(4):
                c = 4 * j + r
                y[i * BC + c * CHUNK:i * BC + (c + 1) * CHUNK, :] = \
                    t[32 * r:32 * r + 10, :].T
    return y


def kernel(**inputs: np.ndarray) -> np.ndarray:
    from concourse.bass_utils import run_bass_kernel_spmd

    plan = Plan(inputs)
    wb, bias = plan.build_weights(inputs)
    nc = build_nc(plan.n1, plan.offs, plan.wcols)
    in_maps = make_in_maps(inputs, plan, wb, bias)
    res = run_bass_kernel_spmd(nc, in_maps, list(range(N_CORES)))
    return assemble_output(res.results)
